# revision 23
# baseline (speedup 1.0000x reference)
"""BEV pooling (Lift-Splat-Shoot scatter) Trainium2 kernel, v3.

Strategy (8 NeuronCores = 4 batches x 2 cell-range shards):
  Geometry structure (identity rots/post_rots in this problem): the BEV cell
  of a frustum point depends only on (d, w); the z-keep mask only on (d, h).
  So per batch: h-reduce x[d,:,w,:] over kept h rows -> S1[(d,w), 80], then
  scatter-add ~9.4K columns into the occupied subset of the 360x360x80 grid.

  v3 (vs v1 300.9us, v2 147.5us):
    - x shipped bf16 (halves input DMA; ~1e-3 of the 2e-2 rel-err budget)
    - rank-space scatter: each shard's occupied cells enumerated densely
      (ranks 0..R-1); device output is compact strips; host places columns
      into the np.zeros canvas (pure permutation - all sums on device)
    - per-core window segmentation against a shared tile-budget sequence T
      (SPMD): NT ~= ideal ceil(cols/128); windows are single PSUM banks
      (SPAN=512 ranks)
    - h-reduce as an in-place bf16 tensor_tensor halving tree: DVE runs
      all-bf16 tensor_tensor in 2x_1p mode (0.52 ns/elem) vs tensor_reduce
      which has NO fast mode (1.04); a fraction of tiles go to the GpSimd
      tree to balance engine time
    - narrow one-hots: 128 consecutive columns cover <=128 dense ranks, so
      tiles k>=1 use a 128-wide one-hot and a 128-col matmul at a per-tile
      offset inside the window bank (offset clamped to SPAN-128); tile 0
      uses a full 512-wide one-hot with start=True to zero the bank

  Device per tile: DMA x-tile [128, 2560] bf16 ([h][c] lane layout);
  5-level halving tree (DVE 2x or GpSimd) -> S1 bf16 in xt[:, :80];
  one-hot = is_equal(iota16, idx) -> bf16; TensorE matmul accumulates
  S1.T @ onehot into the window's PSUM bank. Per window: ScalarE copies
  PSUM -> SBUF strip, DMA out.
"""

import numpy as np

# ---------------- problem constants (hardcoded, self-contained) -------------
B, N = 4, 1
IH, IW = 256, 704
FH, FW = 32, 88
C = 80
XB = (-54.0, 54.0, 0.3)
YB = (-54.0, 54.0, 0.3)
ZB = (-10.0, 10.0, 20.0)
DB = (1.0, 60.0, 0.5)
D = int((DB[1] - DB[0]) / DB[2])          # 118
NXG = (360, 360, 1)
NCELL = NXG[0] * NXG[1]                    # 129600 cells per batch
SPAN = 512                                 # window width in rank space (1 PSUM bank)
HC = FH * C                                # 2560
# per-tile reduce strategy mix (see module docstring):
#   'A': DVE tensor_reduce of the whole tile ([c][h] layout), ~2.8us DVE
#   'C': GpSimd adds halves ([h][c] layout, f32 out), DVE reduces the
#        remaining 16 h values via a strided view, ~2.9us GPS + ~1.5us DVE
FRAC_A = 11 / 38                           # fraction of tiles on strategy A


def _geometry(inputs):
    """Frustum -> lidar-frame points, replicated from the reference.
    jax-on-CPU when available (bit-identical to the reference); numpy
    fallback (verified cell-identical on CPU)."""
    args = [np.asarray(inputs[k]) for k in
            ('rots', 'trans', 'intrins', 'post_rots', 'post_trans',
             'lidar2ego_rots', 'lidar2ego_trans', 'extra_rots', 'extra_trans')]
    try:
        import jax
        import jax.numpy as jnp
        cpu = jax.devices("cpu")[0]
        with jax.default_device(cpu):
            ds_ = jnp.broadcast_to(jnp.arange(DB[0], DB[1], DB[2], dtype=jnp.float32)[:, None, None], (D, FH, FW))
            xs = jnp.broadcast_to(jnp.linspace(0.0, IW - 1.0, FW, dtype=jnp.float32)[None, None, :], (D, FH, FW))
            ys = jnp.broadcast_to(jnp.linspace(0.0, IH - 1.0, FH, dtype=jnp.float32)[None, :, None], (D, FH, FW))
            frustum = jnp.stack([xs, ys, ds_], axis=-1)
            rots, trans, intrins, post_rots, post_trans, l2c_rots, l2c_trans, extra_rots, extra_trans = map(jnp.asarray, args)
            pts = frustum[None, None] - post_trans[:, :, None, None, None, :]
            pts = jnp.einsum('bnij,bndhwj->bndhwi', jnp.linalg.inv(post_rots), pts)
            pts = jnp.concatenate([pts[..., :2] * pts[..., 2:3], pts[..., 2:3]], axis=-1)
            combine = jnp.einsum('bnij,bnjk->bnik', rots, jnp.linalg.inv(intrins))
            pts = jnp.einsum('bnij,bndhwj->bndhwi', combine, pts) + trans[:, :, None, None, None, :]
            pts = pts - l2c_trans[:, None, None, None, None, :]
            pts = jnp.einsum('bij,bndhwj->bndhwi', jnp.linalg.inv(l2c_rots), pts)
            pts = jnp.einsum('bij,bndhwj->bndhwi', extra_rots, pts) + extra_trans[:, None, None, None, None, :]
            return np.asarray(pts)
    except Exception:
        pass
    rots, trans, intrins, post_rots, post_trans, l2c_rots, l2c_trans, extra_rots, extra_trans = \
        [a.astype(np.float32) for a in args]
    ds_ = np.broadcast_to(np.arange(DB[0], DB[1], DB[2], dtype=np.float32)[:, None, None], (D, FH, FW))
    xs = np.broadcast_to(np.linspace(0.0, IW - 1.0, FW, dtype=np.float32)[None, None, :], (D, FH, FW))
    ys = np.broadcast_to(np.linspace(0.0, IH - 1.0, FH, dtype=np.float32)[None, :, None], (D, FH, FW))
    frustum = np.stack([xs, ys, ds_], axis=-1)
    pts = frustum[None, None] - post_trans[:, :, None, None, None, :]
    pts = np.einsum('bnij,bndhwj->bndhwi', np.linalg.inv(post_rots), pts)
    pts = np.concatenate([pts[..., :2] * pts[..., 2:3], pts[..., 2:3]], axis=-1)
    combine = np.einsum('bnij,bnjk->bnik', rots, np.linalg.inv(intrins))
    pts = np.einsum('bnij,bndhwj->bndhwi', combine, pts) + trans[:, :, None, None, None, :]
    pts = pts - l2c_trans[:, None, None, None, None, :]
    pts = np.einsum('bij,bndhwj->bndhwi', np.linalg.inv(l2c_rots), pts)
    pts = np.einsum('bij,bndhwj->bndhwi', extra_rots, pts) + extra_trans[:, None, None, None, None, :]
    return pts.astype(np.float32)


def _greedy_windows(ranks, budgets):
    """Segment a sorted rank list into windows: window w takes at most
    budgets[w]*128 columns, spans < SPAN ranks, and never splits a cell.
    Returns [(i0, i1, r0)] per window (column range, start rank) or None if
    the columns don't fit in len(budgets) windows."""
    segs = []
    i, n = 0, len(ranks)
    for t in budgets:
        if i >= n:
            segs.append((i, i, 0))
            continue
        r0 = ranks[i]
        j = int(np.searchsorted(ranks, r0 + SPAN, side='left'))
        j = min(j, i + t * 128, n)
        while j < n and j > i and ranks[j] == ranks[j - 1]:
            j -= 1
        segs.append((i, j, int(r0)))
        i = j
    return segs if i >= n else None


def kernel(**inputs) -> np.ndarray:
    import os
    import concourse.mybir as mybir
    import concourse.tile as tile
    from concourse import bacc
    from concourse.bass_utils import run_bass_kernel_spmd

    x = np.asarray(inputs['x'])

    # ---------------- host planning: masks, shards, ranks, windows ----------
    geom = _geometry(inputs)                                   # [B,1,D,FH,FW,3]
    DXv = np.array([XB[2], YB[2], ZB[2]], np.float32)
    BXv = np.array([XB[0] + XB[2] / 2, YB[0] + YB[2] / 2, ZB[0] + ZB[2] / 2], np.float32)
    coords = ((geom - (BXv - DXv / 2.0)) / DXv).astype(np.int32)

    cxy = coords[:, 0, :, 0, :, :2]                            # [B, D, FW] (h-indep)
    cz = coords[:, 0, :, :, 0, 2]                              # [B, D, FH] (w-indep)
    assert (coords[..., 0] == coords[:, :, :, :1, :, 0]).all()
    assert (coords[..., 1] == coords[:, :, :, :1, :, 1]).all()
    assert (coords[..., 2] == coords[:, :, :, :, :1, 2]).all()

    xym = ((cxy[..., 0] >= 0) & (cxy[..., 0] < NXG[0]) &
           (cxy[..., 1] >= 0) & (cxy[..., 1] < NXG[1]))        # [B, D, FW]
    zm = (cz == 0)                                             # [B, D, FH]

    # per shard: sorted column list (by cell), dense cell ranks
    shards = []                                                # (dk, wk, ranks, cells)
    for b in range(B):
        dk, wk = np.nonzero(xym[b])
        cx = cxy[b, dk, wk, 0].astype(np.int64)
        cy = cxy[b, dk, wk, 1].astype(np.int64)
        lin = cy * NXG[0] + cx                                 # out[b] flat idx (C, y, x)
        order = np.argsort(lin, kind='stable')
        lin, dk, wk = lin[order], dk[order], wk[order]
        mid = len(lin) // 2
        while mid < len(lin) and lin[mid] == lin[mid - 1]:
            mid += 1
        for sl in (slice(0, mid), slice(mid, None)):
            ls = lin[sl]
            cells, inv = np.unique(ls, return_inverse=True)
            shards.append((dk[sl], wk[sl], inv.astype(np.int64), cells))

    # shared per-window tile budget sequence T: for each uniform seed budget
    # a, iterate T <- elementwise max of per-core greedy packings to a
    # (descending, feasibility-preserving) fixpoint; keep the smallest sum.
    def _fit(budgets):
        seqs = []
        for (_, _, ranks, _) in shards:
            segs = _greedy_windows(ranks, budgets)
            if segs is None:
                return None
            seqs.append([-(-(j - i) // 128) for (i, j, _) in segs])
        return seqs

    best = None
    for a in range(12, 1, -1):
        Tc = [a] * 128
        seqs = _fit(Tc)
        if seqs is None:
            continue
        for _ in range(12):
            Tn = [max(s[w] for s in seqs) for w in range(len(Tc))]
            if Tn == Tc:
                break
            s2 = _fit(Tn)
            if s2 is None:
                break
            Tc, seqs = Tn, s2
        while Tc and Tc[-1] == 0:
            Tc.pop()
        if Tc and (best is None or sum(Tc) < sum(best)):
            best = list(Tc)
    T = best
    NT = sum(T)
    NWIN = len(T)

    # final per-core segmentation against the shared budgets
    plans = []
    for (dk, wk, ranks, cells) in shards:
        segs = _greedy_windows(ranks, T)
        assert segs is not None, "shared window budgets infeasible"
        plans.append(segs)

    # ---------------- pack device inputs ------------------------------------
    # tiles are processed in PAIRS (super-tiles): one DMA / one GpSimd op /
    # one DVE reduce covers two 128-column tiles, halving instruction and
    # semaphore counts. Both tiles of a pair share a reduce strategy.
    assert NT % 2 == 0
    NST = NT // 2
    n_a = max(1, round(FRAC_A * NT))
    strat = ['A' if (ti * n_a) % NT < n_a else 'C' for ti in range(NT)]

    bf16 = mybir.dt.np(mybir.dt.bfloat16)
    fp8 = mybir.dt.np(mybir.dt.float8e4)
    x_perm = np.zeros((8, NST, 128, 2 * HC), dtype=bf16)
    oh_perm = np.zeros((8, NST, 128, 2 * SPAN), dtype=fp8)
    one8 = np.ones((), dtype=fp8)
    xf = x.reshape(B, D, FH, FW, C)
    for s in range(8):
        b = s // 2
        dk, wk, ranks, cells = shards[s]
        zmb = zm[b]
        ti = 0
        for w, t in enumerate(T):
            i0, i1, r0 = plans[s][w]
            for k in range(t):
                lo = i0 + k * 128
                hi = min(i0 + (k + 1) * 128, i1)
                nl = max(0, hi - lo)
                if nl > 0:
                    dsel = dk[lo:hi]
                    wsel = wk[lo:hi]
                    blk = xf[b, dsel, :, wsel, :]              # [nl, FH, C]
                    blk = blk * zmb[dsel][:, :, None]
                    st, g = ti // 2, ti % 2
                    # [c][h] lanes for every strategy
                    x_perm[s, st, :nl, g * HC:(g + 1) * HC] = \
                        blk.transpose(0, 2, 1).reshape(nl, HC).astype(bf16)
                    oh_perm[s, st, np.arange(nl), g * SPAN + ranks[lo:hi] - r0] = one8
                ti += 1
        assert ti == NT

    # ---------------- device program ----------------------------------------
    F32, BF16, FP8 = mybir.dt.float32, mybir.dt.bfloat16, mybir.dt.float8e4
    nc = bacc.Bacc("TRN2", target_bir_lowering=False, debug=False)
    x_d = nc.dram_tensor("xp", [NST, 128, 2 * HC], BF16, kind="ExternalInput").ap()
    oh_d = nc.dram_tensor("oh", [NST, 128, 2 * SPAN], FP8, kind="ExternalInput").ap()
    out_d = nc.dram_tensor("out", [C, NWIN * SPAN], F32, kind="ExternalOutput").ap()

    # window / position of each flat tile index
    w_of, k_of = [], []
    for w, t in enumerate(T):
        for k in range(t):
            w_of.append(w)
            k_of.append(k)

    with tile.TileContext(nc) as tc:
        with (
            tc.tile_pool(name="xt", bufs=6) as xpool,
            tc.tile_pool(name="oh", bufs=6) as ohpool,
            tc.tile_pool(name="red", bufs=4) as redpool,
            tc.tile_pool(name="s1", bufs=8) as s1pool,
            tc.tile_pool(name="strip", bufs=3) as stpool,
            tc.tile_pool(name="psum", bufs=8, space="PSUM") as pspool,
        ):
            with nc.allow_low_precision(reason="bf16 S1 for the scatter matmul; validated vs fp32 reference"):
                ps_of = {}
                for st in range(NST):
                    xt = xpool.tile([128, 2 * HC], BF16, tag="xt")
                    oh = ohpool.tile([128, 2 * SPAN], FP8, tag="oh")
                    lq = nc.sync if st % 2 == 0 else nc.scalar
                    lq.dma_start(oh[:], oh_d[st])
                    lq.dma_start(xt[:], x_d[st])
                    for g in range(2):
                        ti = 2 * st + g
                        xs = xt[:, g * HC:(g + 1) * HC]
                        s1b = s1pool.tile([128, C], BF16, tag="s1b", name=f"s1b{ti}")
                        if strat[ti] == 'A':
                            nc.vector.tensor_reduce(
                                out=s1b[:],
                                in_=xs.rearrange("p (c h) -> p c h", h=FH),
                                axis=mybir.AxisListType.X, op=mybir.AluOpType.add)
                        else:
                            red = redpool.tile([128, HC // 2], BF16, tag="red",
                                               name=f"red{ti}")
                            xv = xs.rearrange("p (c hh h) -> p c hh h",
                                              hh=2, h=FH // 2)
                            nc.gpsimd.tensor_tensor(
                                out=red[:].rearrange("p (c h) -> p c h", h=FH // 2),
                                in0=xv[:, :, 0, :], in1=xv[:, :, 1, :],
                                op=mybir.AluOpType.add)
                            nc.vector.tensor_reduce(
                                out=s1b[:],
                                in_=red[:].rearrange("p (c h) -> p c h", h=FH // 2),
                                axis=mybir.AxisListType.X, op=mybir.AluOpType.add)
                        w, k, t = w_of[ti], k_of[ti], T[w_of[ti]]
                        if k == 0:
                            ps_of[w] = pspool.tile([C, SPAN], F32, tag="ps",
                                                   name=f"ps{w}")
                        nc.tensor.matmul(out=ps_of[w][:],
                                         lhsT=s1b[:],
                                         rhs=oh[:, g * SPAN:(g + 1) * SPAN],
                                         start=(k == 0), stop=(k == t - 1))
                        if k == t - 1:
                            strip = stpool.tile([C, SPAN], F32, tag="strip",
                                                name=f"strip{w}")
                            nc.scalar.activation(out=strip[:], in_=ps_of[w][:],
                                                 func=mybir.ActivationFunctionType.Copy)
                            nc.sync.dma_start(out_d[:, w * SPAN:(w + 1) * SPAN], strip[:])
    nc.compile()

    # ---------------- run on 8 cores, place strips into the canvas ----------
    in_maps = [{"xp": x_perm[s], "oh": oh_perm[s]} for s in range(8)]
    trace = os.environ.get("KERNEL_TRACE", "") == "1"
    res = run_bass_kernel_spmd(nc, in_maps, core_ids=list(range(8)), trace=trace)
    et = getattr(res, "exec_time_ns", None)
    if et is not None:
        globals()["LAST_EXEC_TIME_NS"] = et
        it = getattr(res, "instructions_and_trace", None)
        globals()["LAST_TRACE_PATH"] = it[1] if it else None

    out = np.zeros((B, C, NXG[1], NXG[0]), np.float32)
    for s in range(8):
        b = s // 2
        _, _, ranks, cells = shards[s]
        flat = out[b].reshape(C, NCELL)
        strip = res.results[s]["out"]                          # [C, NWIN*SPAN]
        for w in range(NWIN):
            i0, i1, r0 = plans[s][w]
            if i1 > i0:
                r1 = int(ranks[i1 - 1]) + 1
                flat[:, cells[r0:r1]] = strip[:, w * SPAN: w * SPAN + (r1 - r0)]
    return out


# revision 25
# speedup vs baseline: 1.2061x; 1.2061x over previous
"""BEV pooling (Lift-Splat-Shoot scatter) Trainium2 kernel, v3.

Strategy (8 NeuronCores = 4 batches x 2 cell-range shards):
  Geometry structure (identity rots/post_rots in this problem): the BEV cell
  of a frustum point depends only on (d, w); the z-keep mask only on (d, h).
  So per batch: h-reduce x[d,:,w,:] over kept h rows -> S1[(d,w), 80], then
  scatter-add ~9.4K columns into the occupied subset of the 360x360x80 grid.

  v3 (vs v1 300.9us, v2 147.5us):
    - x shipped bf16 (halves input DMA; ~1e-3 of the 2e-2 rel-err budget)
    - rank-space scatter: each shard's occupied cells enumerated densely
      (ranks 0..R-1); device output is compact strips; host places columns
      into the np.zeros canvas (pure permutation - all sums on device)
    - per-core window segmentation against a shared tile-budget sequence T
      (SPMD): NT ~= ideal ceil(cols/128); windows are single PSUM banks
      (SPAN=512 ranks)
    - h-reduce as an in-place bf16 tensor_tensor halving tree: DVE runs
      all-bf16 tensor_tensor in 2x_1p mode (0.52 ns/elem) vs tensor_reduce
      which has NO fast mode (1.04); a fraction of tiles go to the GpSimd
      tree to balance engine time
    - narrow one-hots: 128 consecutive columns cover <=128 dense ranks, so
      tiles k>=1 use a 128-wide one-hot and a 128-col matmul at a per-tile
      offset inside the window bank (offset clamped to SPAN-128); tile 0
      uses a full 512-wide one-hot with start=True to zero the bank

  Device per tile: DMA x-tile [128, 2560] bf16 ([h][c] lane layout);
  5-level halving tree (DVE 2x or GpSimd) -> S1 bf16 in xt[:, :80];
  one-hot = is_equal(iota16, idx) -> bf16; TensorE matmul accumulates
  S1.T @ onehot into the window's PSUM bank. Per window: ScalarE copies
  PSUM -> SBUF strip, DMA out.
"""

import numpy as np

# ---------------- problem constants (hardcoded, self-contained) -------------
B, N = 4, 1
IH, IW = 256, 704
FH, FW = 32, 88
C = 80
XB = (-54.0, 54.0, 0.3)
YB = (-54.0, 54.0, 0.3)
ZB = (-10.0, 10.0, 20.0)
DB = (1.0, 60.0, 0.5)
D = int((DB[1] - DB[0]) / DB[2])          # 118
NXG = (360, 360, 1)
NCELL = NXG[0] * NXG[1]                    # 129600 cells per batch
SPAN = 512                                 # window width in rank space (1 PSUM bank)
HC = FH * C                                # 2560
# per-tile reduce strategy mix (see module docstring):
#   'A': DVE tensor_reduce of the whole tile ([c][h] layout), ~2.8us DVE
#   'C': GpSimd adds halves ([h][c] layout, f32 out), DVE reduces the
#        remaining 16 h values via a strided view, ~2.9us GPS + ~1.5us DVE
FRAC_A = 11 / 38                           # fraction of tiles on strategy A


def _geometry(inputs):
    """Frustum -> lidar-frame points, replicated from the reference.
    jax-on-CPU when available (bit-identical to the reference); numpy
    fallback (verified cell-identical on CPU)."""
    args = [np.asarray(inputs[k]) for k in
            ('rots', 'trans', 'intrins', 'post_rots', 'post_trans',
             'lidar2ego_rots', 'lidar2ego_trans', 'extra_rots', 'extra_trans')]
    try:
        import jax
        import jax.numpy as jnp
        cpu = jax.devices("cpu")[0]
        with jax.default_device(cpu):
            ds_ = jnp.broadcast_to(jnp.arange(DB[0], DB[1], DB[2], dtype=jnp.float32)[:, None, None], (D, FH, FW))
            xs = jnp.broadcast_to(jnp.linspace(0.0, IW - 1.0, FW, dtype=jnp.float32)[None, None, :], (D, FH, FW))
            ys = jnp.broadcast_to(jnp.linspace(0.0, IH - 1.0, FH, dtype=jnp.float32)[None, :, None], (D, FH, FW))
            frustum = jnp.stack([xs, ys, ds_], axis=-1)
            rots, trans, intrins, post_rots, post_trans, l2c_rots, l2c_trans, extra_rots, extra_trans = map(jnp.asarray, args)
            pts = frustum[None, None] - post_trans[:, :, None, None, None, :]
            pts = jnp.einsum('bnij,bndhwj->bndhwi', jnp.linalg.inv(post_rots), pts)
            pts = jnp.concatenate([pts[..., :2] * pts[..., 2:3], pts[..., 2:3]], axis=-1)
            combine = jnp.einsum('bnij,bnjk->bnik', rots, jnp.linalg.inv(intrins))
            pts = jnp.einsum('bnij,bndhwj->bndhwi', combine, pts) + trans[:, :, None, None, None, :]
            pts = pts - l2c_trans[:, None, None, None, None, :]
            pts = jnp.einsum('bij,bndhwj->bndhwi', jnp.linalg.inv(l2c_rots), pts)
            pts = jnp.einsum('bij,bndhwj->bndhwi', extra_rots, pts) + extra_trans[:, None, None, None, None, :]
            return np.asarray(pts)
    except Exception:
        pass
    rots, trans, intrins, post_rots, post_trans, l2c_rots, l2c_trans, extra_rots, extra_trans = \
        [a.astype(np.float32) for a in args]
    ds_ = np.broadcast_to(np.arange(DB[0], DB[1], DB[2], dtype=np.float32)[:, None, None], (D, FH, FW))
    xs = np.broadcast_to(np.linspace(0.0, IW - 1.0, FW, dtype=np.float32)[None, None, :], (D, FH, FW))
    ys = np.broadcast_to(np.linspace(0.0, IH - 1.0, FH, dtype=np.float32)[None, :, None], (D, FH, FW))
    frustum = np.stack([xs, ys, ds_], axis=-1)
    pts = frustum[None, None] - post_trans[:, :, None, None, None, :]
    pts = np.einsum('bnij,bndhwj->bndhwi', np.linalg.inv(post_rots), pts)
    pts = np.concatenate([pts[..., :2] * pts[..., 2:3], pts[..., 2:3]], axis=-1)
    combine = np.einsum('bnij,bnjk->bnik', rots, np.linalg.inv(intrins))
    pts = np.einsum('bnij,bndhwj->bndhwi', combine, pts) + trans[:, :, None, None, None, :]
    pts = pts - l2c_trans[:, None, None, None, None, :]
    pts = np.einsum('bij,bndhwj->bndhwi', np.linalg.inv(l2c_rots), pts)
    pts = np.einsum('bij,bndhwj->bndhwi', extra_rots, pts) + extra_trans[:, None, None, None, None, :]
    return pts.astype(np.float32)


def _greedy_windows(ranks, budgets):
    """Segment a sorted rank list into windows: window w takes at most
    budgets[w]*128 columns, spans < SPAN ranks, and never splits a cell.
    Returns [(i0, i1, r0)] per window (column range, start rank) or None if
    the columns don't fit in len(budgets) windows."""
    segs = []
    i, n = 0, len(ranks)
    for t in budgets:
        if i >= n:
            segs.append((i, i, 0))
            continue
        r0 = ranks[i]
        j = int(np.searchsorted(ranks, r0 + SPAN, side='left'))
        j = min(j, i + t * 128, n)
        while j < n and j > i and ranks[j] == ranks[j - 1]:
            j -= 1
        segs.append((i, j, int(r0)))
        i = j
    return segs if i >= n else None


def kernel(**inputs) -> np.ndarray:
    import os
    import concourse.mybir as mybir
    import concourse.tile as tile
    from concourse import bacc
    from concourse.bass_utils import run_bass_kernel_spmd

    x = np.asarray(inputs['x'])

    # ---------------- host planning: masks, shards, ranks, windows ----------
    geom = _geometry(inputs)                                   # [B,1,D,FH,FW,3]
    DXv = np.array([XB[2], YB[2], ZB[2]], np.float32)
    BXv = np.array([XB[0] + XB[2] / 2, YB[0] + YB[2] / 2, ZB[0] + ZB[2] / 2], np.float32)
    coords = ((geom - (BXv - DXv / 2.0)) / DXv).astype(np.int32)

    cxy = coords[:, 0, :, 0, :, :2]                            # [B, D, FW] (h-indep)
    cz = coords[:, 0, :, :, 0, 2]                              # [B, D, FH] (w-indep)
    assert (coords[..., 0] == coords[:, :, :, :1, :, 0]).all()
    assert (coords[..., 1] == coords[:, :, :, :1, :, 1]).all()
    assert (coords[..., 2] == coords[:, :, :, :, :1, 2]).all()

    xym = ((cxy[..., 0] >= 0) & (cxy[..., 0] < NXG[0]) &
           (cxy[..., 1] >= 0) & (cxy[..., 1] < NXG[1]))        # [B, D, FW]
    zm = (cz == 0)                                             # [B, D, FH]

    # per shard: sorted column list (by cell), dense cell ranks
    shards = []                                                # (dk, wk, ranks, cells)
    for b in range(B):
        dk, wk = np.nonzero(xym[b])
        cx = cxy[b, dk, wk, 0].astype(np.int64)
        cy = cxy[b, dk, wk, 1].astype(np.int64)
        lin = cy * NXG[0] + cx                                 # out[b] flat idx (C, y, x)
        order = np.argsort(lin, kind='stable')
        lin, dk, wk = lin[order], dk[order], wk[order]
        mid = len(lin) // 2
        while mid < len(lin) and lin[mid] == lin[mid - 1]:
            mid += 1
        for sl in (slice(0, mid), slice(mid, None)):
            ls = lin[sl]
            cells, inv = np.unique(ls, return_inverse=True)
            shards.append((dk[sl], wk[sl], inv.astype(np.int64), cells))

    # shared per-window tile budget sequence T: for each uniform seed budget
    # a, iterate T <- elementwise max of per-core greedy packings to a
    # (descending, feasibility-preserving) fixpoint; keep the smallest sum.
    def _fit(budgets):
        seqs = []
        for (_, _, ranks, _) in shards:
            segs = _greedy_windows(ranks, budgets)
            if segs is None:
                return None
            seqs.append([-(-(j - i) // 128) for (i, j, _) in segs])
        return seqs

    best = None
    for a in range(12, 1, -1):
        Tc = [a] * 128
        seqs = _fit(Tc)
        if seqs is None:
            continue
        for _ in range(12):
            Tn = [max(s[w] for s in seqs) for w in range(len(Tc))]
            if Tn == Tc:
                break
            s2 = _fit(Tn)
            if s2 is None:
                break
            Tc, seqs = Tn, s2
        while Tc and Tc[-1] == 0:
            Tc.pop()
        if Tc and (best is None or sum(Tc) < sum(best)):
            best = list(Tc)
    T = best
    NT = sum(T)
    NWIN = len(T)

    # final per-core segmentation against the shared budgets
    plans = []
    for (dk, wk, ranks, cells) in shards:
        segs = _greedy_windows(ranks, T)
        assert segs is not None, "shared window budgets infeasible"
        plans.append(segs)

    # ---------------- pack device inputs ------------------------------------
    # tiles are processed in PAIRS (super-tiles): one DMA / one GpSimd op /
    # one DVE reduce covers two 128-column tiles, halving instruction and
    # semaphore counts. Both tiles of a pair share a reduce strategy.
    n_a = max(1, round(FRAC_A * NT))
    strat = ['A' if (ti * n_a) % NT < n_a else 'C' for ti in range(NT)]

    bf16 = mybir.dt.np(mybir.dt.bfloat16)
    fp8 = mybir.dt.np(mybir.dt.float8e4)
    x_perm = np.zeros((8, NT, 128, HC), dtype=bf16)
    oh_perm = np.zeros((8, NT, 128, SPAN), dtype=fp8)
    one8 = np.ones((), dtype=fp8)
    xf = x.reshape(B, D, FH, FW, C)
    for s in range(8):
        b = s // 2
        dk, wk, ranks, cells = shards[s]
        zmb = zm[b]
        ti = 0
        for w, t in enumerate(T):
            i0, i1, r0 = plans[s][w]
            for k in range(t):
                lo = i0 + k * 128
                hi = min(i0 + (k + 1) * 128, i1)
                nl = max(0, hi - lo)
                if nl > 0:
                    dsel = dk[lo:hi]
                    wsel = wk[lo:hi]
                    blk = xf[b, dsel, :, wsel, :]              # [nl, FH, C]
                    blk = blk * zmb[dsel][:, :, None]
                    # [c][h] lanes for every strategy
                    x_perm[s, ti, :nl] = blk.transpose(0, 2, 1).reshape(nl, HC).astype(bf16)
                    oh_perm[s, ti, np.arange(nl), ranks[lo:hi] - r0] = one8
                ti += 1
        assert ti == NT

    # ---------------- device program ----------------------------------------
    F32, BF16, FP8 = mybir.dt.float32, mybir.dt.bfloat16, mybir.dt.float8e4
    nc = bacc.Bacc("TRN2", target_bir_lowering=False, debug=False)
    x_d = nc.dram_tensor("xp", [NT, 128, HC], BF16, kind="ExternalInput").ap()
    oh_d = nc.dram_tensor("oh", [NT, 128, SPAN], FP8, kind="ExternalInput").ap()
    out_d = nc.dram_tensor("out", [C, NWIN * SPAN], F32, kind="ExternalOutput").ap()

    with tile.TileContext(nc) as tc:
        with (
            tc.tile_pool(name="xt", bufs=12) as xpool,
            tc.tile_pool(name="oh", bufs=8) as ohpool,
            tc.tile_pool(name="red", bufs=6) as redpool,
            tc.tile_pool(name="s1", bufs=8) as s1pool,
            tc.tile_pool(name="strip", bufs=3) as stpool,
            tc.tile_pool(name="psum", bufs=8, space="PSUM") as pspool,
        ):
            with nc.allow_low_precision(reason="bf16 S1 for the scatter matmul; validated vs fp32 reference"):
                ti = 0
                for w, t in enumerate(T):
                    ps = pspool.tile([C, SPAN], F32, tag="ps", name=f"ps{w}")
                    for k in range(t):
                        xt = xpool.tile([128, HC], BF16, tag="xt")
                        oh = ohpool.tile([128, SPAN], FP8, tag="oh")
                        lq = nc.sync if ti % 2 == 0 else nc.scalar
                        lq.dma_start(oh[:], oh_d[ti])
                        lq.dma_start(xt[:], x_d[ti])
                        s1b = s1pool.tile([128, C], BF16, tag="s1b")
                        if strat[ti] == 'A':
                            nc.vector.tensor_reduce(
                                out=s1b[:],
                                in_=xt[:].rearrange("p (c h) -> p c h", h=FH),
                                axis=mybir.AxisListType.X, op=mybir.AluOpType.add)
                        else:
                            red = redpool.tile([128, HC // 2], BF16, tag="red")
                            xv = xt[:].rearrange("p (c hh h) -> p c hh h", hh=2, h=FH // 2)
                            nc.gpsimd.tensor_tensor(
                                out=red[:].rearrange("p (c h) -> p c h", h=FH // 2),
                                in0=xv[:, :, 0, :], in1=xv[:, :, 1, :],
                                op=mybir.AluOpType.add)
                            nc.vector.tensor_reduce(
                                out=s1b[:],
                                in_=red[:].rearrange("p (c h) -> p c h", h=FH // 2),
                                axis=mybir.AxisListType.X, op=mybir.AluOpType.add)
                        nc.tensor.matmul(out=ps[:], lhsT=s1b[:], rhs=oh[:],
                                         start=(k == 0), stop=(k == t - 1))
                        ti += 1
                    strip = stpool.tile([C, SPAN], F32, tag="strip")
                    nc.scalar.activation(out=strip[:], in_=ps[:],
                                         func=mybir.ActivationFunctionType.Copy)
                    nc.sync.dma_start(out_d[:, w * SPAN:(w + 1) * SPAN], strip[:])
                assert ti == NT
    nc.compile()

    # ---------------- run on 8 cores, place strips into the canvas ----------
    in_maps = [{"xp": x_perm[s], "oh": oh_perm[s]} for s in range(8)]
    trace = os.environ.get("KERNEL_TRACE", "") == "1"
    res = run_bass_kernel_spmd(nc, in_maps, core_ids=list(range(8)), trace=trace)
    et = getattr(res, "exec_time_ns", None)
    if et is not None:
        globals()["LAST_EXEC_TIME_NS"] = et
        it = getattr(res, "instructions_and_trace", None)
        globals()["LAST_TRACE_PATH"] = it[1] if it else None

    out = np.zeros((B, C, NXG[1], NXG[0]), np.float32)
    for s in range(8):
        b = s // 2
        _, _, ranks, cells = shards[s]
        flat = out[b].reshape(C, NCELL)
        strip = res.results[s]["out"]                          # [C, NWIN*SPAN]
        for w in range(NWIN):
            i0, i1, r0 = plans[s][w]
            if i1 > i0:
                r1 = int(ranks[i1 - 1]) + 1
                flat[:, cells[r0:r1]] = strip[:, w * SPAN: w * SPAN + (r1 - r0)]
    return out
